# revision 1
# baseline (speedup 1.0000x reference)
import sys
sys.path.insert(0, "/opt/trn_rl_repo")
import numpy as np
import concourse.bass as bass
import concourse.mybir as mybir
import concourse.tile as tile
from concourse import bacc
from concourse.bass_utils import run_bass_kernel_spmd
from concourse.masks import make_identity

F32 = mybir.dt.float32
F32R = mybir.dt.float32r
AF = mybir.ActivationFunctionType
OP = mybir.AluOpType

S = 2048          # sequence length
H = 4096          # hidden dim
DHEAD = 128       # head dim
NQ = 4            # q heads per core
NCORES = 8
SC = 4            # s-chunks of 512
HO = 32           # h k-tiles of 128
SCALE = 1.0 / np.sqrt(128.0)

_CACHED = {}


def _build_nc():
    nc = bacc.Bacc(None, target_bir_lowering=False, debug=False)
    hid_d = nc.dram_tensor("hidden", [S, H], F32, kind="ExternalInput")
    wqkv_d = nc.dram_tensor("wqkv", [768, H], F32, kind="ExternalInput")
    wo_d = nc.dram_tensor("wo", [H, 512], F32, kind="ExternalInput")
    cos_d = nc.dram_tensor("cos", [128, S], F32, kind="ExternalInput")
    sin_d = nc.dram_tensor("sin", [128, S], F32, kind="ExternalInput")
    out_d = nc.dram_tensor("out", [S, H], F32, kind="ExternalOutput")

    with tile.TileContext(nc) as tc:
        with tc.tile_pool(name="perm", bufs=1) as perm:
            ident = perm.tile([128, 128], F32, tag="ident")
            make_identity(nc, ident)
            identr = perm.tile([128, 128], F32R, tag="identr")
            nc.vector.tensor_copy(identr, ident)
            ones_f = perm.tile([128, 128], F32, tag="ones_f")
            nc.gpsimd.memset(ones_f, 1.0)
            ones = perm.tile([128, 128], F32R, tag="ones")
            nc.vector.tensor_copy(ones, ones_f)
            # qT/kT/vT strips, [d=128, strip, s]: strips 0-3 = Q heads, 4 = K, 5 = V
            strips = perm.tile([128, 6, S], F32R, tag="strips")

            # ---------------- Phase B: projections ----------------
            with tc.tile_pool(name="wtp", bufs=1) as wtp, \
                 tc.tile_pool(name="wn", bufs=2) as wn_p, \
                 tc.tile_pool(name="hp", bufs=2) as hp_p, \
                 tc.tile_pool(name="ht", bufs=3) as ht_p, \
                 tc.tile_pool(name="cs", bufs=2) as cs_p, \
                 tc.tile_pool(name="rt", bufs=2) as rt_p, \
                 tc.tile_pool(name="ppj", bufs=1, space="PSUM") as ppj, \
                 tc.tile_pool(name="ptr", bufs=2, space="PSUM") as ptr:

                # transpose W_qkv [768, H] -> wt [128, HO, 768] (f32r)
                wt = wtp.tile([128, HO, 768], F32R, tag="wt")
                for dt in range(6):
                    for part in range(8):
                        wn = wn_p.tile([128, 512], F32, tag="wn")
                        nc.sync.dma_start(
                            wn, wqkv_d[dt * 128:(dt + 1) * 128, part * 512:(part + 1) * 512])
                        pt4 = ptr.tile([128, 512], F32, tag="tp")
                        for j in range(4):
                            nc.tensor.transpose(
                                pt4[:, j * 128:(j + 1) * 128],
                                wn[:, j * 128:(j + 1) * 128], ident)
                        nc.scalar.copy(
                            wt[:, part * 4:(part + 1) * 4, dt * 128:(dt + 1) * 128],
                            pt4.rearrange("p (a b) -> p a b", a=4))

                for sc in range(SC):
                    psums = [ppj.tile([128, 512], F32, tag=f"pj{d}", name=f"pj{d}") for d in range(6)]
                    for part in range(16):  # h pieces of 256
                        hps = []
                        for st4 in range(4):
                            hp = hp_p.tile([128, 256], F32, tag=f"hp{st4}")
                            nc.sync.dma_start(
                                hp, hid_d[sc * 512 + st4 * 128: sc * 512 + (st4 + 1) * 128,
                                          part * 256:(part + 1) * 256])
                            hps.append(hp)
                        for j in range(2):
                            ho = part * 2 + j
                            ht = ht_p.tile([128, 512], F32R, tag="ht")
                            pt4 = ptr.tile([128, 512], F32, tag="tp")
                            for st4 in range(4):
                                nc.tensor.transpose(
                                    pt4[:, st4 * 128:(st4 + 1) * 128],
                                    hps[st4][:, j * 128:(j + 1) * 128], ident)
                            nc.scalar.copy(ht, pt4)
                            for d in range(6):
                                nc.tensor.matmul(
                                    psums[d], wt[:, ho, d * 128:(d + 1) * 128], ht,
                                    start=(ho == 0), stop=(ho == HO - 1))
                    # RoPE (strips 0-4) / copy (strip 5 = V)
                    cos_c = cs_p.tile([128, 512], F32, tag="cosc")
                    sin_c = cs_p.tile([128, 512], F32, tag="sinc")
                    nc.sync.dma_start(cos_c, cos_d[:, sc * 512:(sc + 1) * 512])
                    nc.sync.dma_start(sin_c, sin_d[:, sc * 512:(sc + 1) * 512])
                    for d in range(6):
                        dst = strips[:, d, sc * 512:(sc + 1) * 512]
                        if d < 5:
                            t1 = rt_p.tile([128, 512], F32, tag="t1")
                            t2 = rt_p.tile([128, 512], F32, tag="t2")
                            nc.vector.tensor_mul(t1, psums[d], cos_c)
                            nc.vector.tensor_mul(t2[0:64], psums[d][64:128], sin_c[0:64])
                            nc.vector.tensor_mul(t2[64:128], psums[d][0:64], sin_c[64:128])
                            nc.vector.tensor_add(dst, t1, t2)
                        else:
                            nc.scalar.copy(dst, psums[d])

            # ---------------- Phase C: attention ----------------
            with tc.tile_pool(name="perm2", bufs=1) as perm2:
                vnat = perm2.tile([128, 16, 128], F32R, tag="vnat")
                attnT = perm2.tile([128, NQ, S], F32R, tag="attnT")
                ones_m = perm2.tile([128, 512], F32, tag="ones_m")
                nc.gpsimd.memset(ones_m, 1.0)
                masks = perm2.tile([128, 4, 512], F32, tag="masks")
                for j in range(4):
                    nc.gpsimd.affine_select(
                        out=masks[:, j, :], in_=ones_m, pattern=[[1, 512]],
                        compare_op=OP.is_ge, fill=0.0,
                        base=-128 * j, channel_multiplier=-1)
                with tc.tile_pool(name="pts", bufs=4) as pts_p, \
                     tc.tile_pool(name="rec", bufs=2) as rec_p, \
                     tc.tile_pool(name="ps_s", bufs=2, space="PSUM") as s_p, \
                     tc.tile_pool(name="ps_pv", bufs=2, space="PSUM") as pv_p, \
                     tc.tile_pool(name="ps_dn", bufs=2, space="PSUM") as dn_p, \
                     tc.tile_pool(name="ptrc", bufs=2, space="PSUM") as ptrc:
                    # V natural tiles from V^T strip
                    for g in range(4):
                        pt4 = ptrc.tile([128, 512], F32R, tag="tpc")
                        for i in range(4):
                            st = 4 * g + i
                            nc.tensor.transpose(
                                pt4[:, i * 128:(i + 1) * 128],
                                strips[:, 5, st * 128:(st + 1) * 128], identr)
                        nc.vector.tensor_copy(
                            vnat[:, 4 * g:4 * g + 4, :],
                            pt4.rearrange("p (a b) -> p a b", a=4))

                    for h in range(NQ):
                        for c in range(SC):
                            nkt = 4 * c + 4
                            pv = pv_p.tile([128, 512], F32, tag="pv")
                            den = dn_p.tile([128, 512], F32, tag="den")
                            for kt in range(nkt):
                                sp = s_p.tile([128, 512], F32, tag="s")
                                nc.tensor.matmul(
                                    sp, strips[:, 4, kt * 128:(kt + 1) * 128],
                                    strips[:, h, c * 512:(c + 1) * 512],
                                    start=True, stop=True)
                                ptile = pts_p.tile([128, 512], F32R, tag="pt")
                                nc.scalar.activation(ptile, sp, AF.Exp, scale=SCALE)
                                j = kt - 4 * c
                                if j >= 0:
                                    nc.vector.tensor_mul(ptile, ptile, masks[:, j, :])
                                nc.tensor.matmul(pv, vnat[:, kt, :], ptile,
                                                 start=(kt == 0), stop=(kt == nkt - 1))
                                nc.tensor.matmul(den, ones, ptile,
                                                 start=(kt == 0), stop=(kt == nkt - 1))
                            rec = rec_p.tile([128, 512], F32, tag="rec")
                            nc.vector.reciprocal(rec, den)
                            nc.vector.tensor_mul(
                                attnT[:, h, c * 512:(c + 1) * 512], pv, rec)

                # ---------------- Phase D: o_proj ----------------
                with tc.tile_pool(name="wotp", bufs=1) as wotp, \
                     tc.tile_pool(name="pos", bufs=4) as pos_p, \
                     tc.tile_pool(name="won", bufs=2) as won_p, \
                     tc.tile_pool(name="po", bufs=4, space="PSUM") as po_p, \
                     tc.tile_pool(name="ptrd", bufs=2, space="PSUM") as ptrd:
                    woT = wotp.tile([128, NQ, H], F32R, tag="woT")
                    for mt in range(32):
                        wn = won_p.tile([128, 512], F32, tag="won")
                        nc.sync.dma_start(wn, wo_d[mt * 128:(mt + 1) * 128, :])
                        pt4 = ptrd.tile([128, 512], F32, tag="tpd")
                        for at in range(4):
                            nc.tensor.transpose(
                                pt4[:, at * 128:(at + 1) * 128],
                                wn[:, at * 128:(at + 1) * 128], ident)
                        nc.scalar.copy(woT[:, :, mt * 128:(mt + 1) * 128],
                                       pt4.rearrange("p (a b) -> p a b", a=4))
                    for st in range(16):
                        for mc in range(8):
                            po = po_p.tile([128, 512], F32, tag="po")
                            for at in range(4):
                                nc.tensor.matmul(
                                    po, attnT[:, at, st * 128:(st + 1) * 128],
                                    woT[:, at, mc * 512:(mc + 1) * 512],
                                    start=(at == 0), stop=(at == 3))
                            pos = pos_p.tile([128, 512], F32, tag="pos")
                            nc.vector.tensor_copy(pos, po)
                            nc.sync.dma_start(
                                out_d[st * 128:(st + 1) * 128, mc * 512:(mc + 1) * 512],
                                pos)
    nc.compile()
    return nc


def kernel(hidden_states, position_ids, Wq, Wk, Wv, Wo, **extra):
    hidden_states = np.asarray(hidden_states, dtype=np.float32)
    position_ids = np.asarray(position_ids)
    Wq = np.asarray(Wq, dtype=np.float32)
    Wk = np.asarray(Wk, dtype=np.float32)
    Wv = np.asarray(Wv, dtype=np.float32)
    Wo = np.asarray(Wo, dtype=np.float32)
    B = hidden_states.shape[0]
    assert B == 1 and hidden_states.shape[1] == S and hidden_states.shape[2] == H

    if "nc" not in _CACHED:
        _CACHED["nc"] = _build_nc()
    nc = _CACHED["nc"]

    # RoPE tables in [d=128, s] layout; sin has sign folded for rotate_half
    pos = position_ids.reshape(-1).astype(np.float64)  # [S]
    invf = 1.0 / (10000.0 ** (np.arange(0, 128, 2, dtype=np.float64) / 128.0))  # [64]
    ang = invf[:, None] * pos[None, :]                 # [64, S]
    cos_t = np.concatenate([np.cos(ang), np.cos(ang)], axis=0).astype(np.float32)
    sin_t = np.concatenate([-np.sin(ang), np.sin(ang)], axis=0).astype(np.float32)

    hid = np.ascontiguousarray(hidden_states[0])
    in_maps = []
    for c in range(NCORES):
        wqkv = np.ascontiguousarray(np.concatenate([
            Wq[c * 512:(c + 1) * 512],
            Wk[c * 128:(c + 1) * 128],
            Wv[c * 128:(c + 1) * 128]], axis=0))
        wo_c = np.ascontiguousarray(Wo[:, c * 512:(c + 1) * 512])
        in_maps.append({"hidden": hid, "wqkv": wqkv, "wo": wo_c,
                        "cos": cos_t, "sin": sin_t})

    res = run_bass_kernel_spmd(nc, in_maps, core_ids=list(range(NCORES)))
    out = np.zeros((S, H), dtype=np.float32)
    for c in range(NCORES):
        out += res.results[c]["out"]
    return out.reshape(1, S, H)



# revision 3
# speedup vs baseline: 1.7835x; 1.7835x over previous
import sys
sys.path.insert(0, "/opt/trn_rl_repo")
import numpy as np
import concourse.bass as bass
import concourse.mybir as mybir
import concourse.tile as tile
from concourse import bacc
from concourse.bass_utils import run_bass_kernel_spmd
from concourse.masks import make_identity

F32 = mybir.dt.float32
BF16 = mybir.dt.bfloat16
AF = mybir.ActivationFunctionType
OP = mybir.AluOpType
AX = mybir.AxisListType

S = 2048          # sequence length
H = 4096          # hidden dim
DH = 128          # head dim
NQ = 4            # q heads per core (32 / 8)
NT = S // 128     # 16 q tiles of 128
NCORES = 8
SCALE = 1.0 / np.sqrt(128.0)
NEG = -1.0e33

_CACHED = {}


def _build_nc():
    nc = bacc.Bacc(None, target_bir_lowering=False, debug=False)
    # All inputs host-pre-transposed/cast so no PE transposes are needed:
    #   hidt  = hidden[0].T            [H, S]
    #   wqkvt = [Wq_c; Wk_c; Wv_c].T   [H, 768]   (cols 0:512 q, 512:640 k, 640:768 v)
    #   wot   = Wo[:, c*512:...].T     [512, H]
    #   cos/sin [d=128, S], sin sign-folded for rotate_half
    hidT_d = nc.dram_tensor("hidt", [H, S], BF16, kind="ExternalInput")
    wqkvT_d = nc.dram_tensor("wqkvt", [H, 768], BF16, kind="ExternalInput")
    woT_d = nc.dram_tensor("wot", [NQ * DH, H], BF16, kind="ExternalInput")
    cos_d = nc.dram_tensor("cos", [DH, S], BF16, kind="ExternalInput")
    sin_d = nc.dram_tensor("sin", [DH, S], BF16, kind="ExternalInput")
    out_d = nc.dram_tensor("outt", [H, S], BF16, kind="ExternalOutput")

    with tile.TileContext(nc) as tc:
        with tc.tile_pool(name="perm", bufs=1) as perm:
            identf = perm.tile([128, 128], F32, tag="identf")
            make_identity(nc, identf)
            identb = perm.tile([128, 128], BF16, tag="identb")
            nc.vector.tensor_copy(identb, identf)
            # additive causal mask for the diagonal 128x128 block:
            # 0 where k <= q, NEG where k > q  (q = partition, k = free)
            zeros = perm.tile([128, 128], F32, tag="zeros")
            nc.gpsimd.memset(zeros, 0.0)
            dmask = perm.tile([128, 128], F32, tag="dmask")
            nc.gpsimd.affine_select(
                out=dmask, in_=zeros, pattern=[[-1, 128]],
                compare_op=OP.is_ge, fill=NEG,
                base=0, channel_multiplier=1)

            # persistent strips (bf16): q^T per head, k^T, v^T, V natural, attn^T
            qT = perm.tile([128, NQ, S], BF16, tag="qT")
            kT = perm.tile([128, S], BF16, tag="kT")
            vT = perm.tile([128, S], BF16, tag="vT")
            vnat = perm.tile([128, NT, 128], BF16, tag="vnat")
            attnT = perm.tile([128, NQ, S], BF16, tag="attnT")
            cosb = perm.tile([128, S], BF16, tag="cosb")
            sinb = perm.tile([128, S], BF16, tag="sinb")
            nc.sync.dma_start(cosb, cos_d[:, :])
            nc.sync.dma_start(sinb, sin_d[:, :])
            # resident W_qkv^T [h=128, kt, 768]
            wq_sb = perm.tile([128, 32, 768], BF16, tag="wq")
            for kt in range(32):
                nc.sync.dma_start(wq_sb[:, kt, :],
                                  wqkvT_d[kt * 128:(kt + 1) * 128, :])

            # ---------------- Phase B: QKV projections + RoPE ----------------
            with tc.tile_pool(name="hid", bufs=2) as hid_p, \
                 tc.tile_pool(name="bps", bufs=2, space="PSUM") as bps, \
                 tc.tile_pool(name="rt", bufs=2) as rt_p:
                for sc in range(4):
                    ssl = slice(sc * 512, (sc + 1) * 512)
                    hid_sb = hid_p.tile([128, 32, 512], BF16, tag="hid")
                    for kt in range(32):
                        nc.sync.dma_start(
                            hid_sb[:, kt, :],
                            hidT_d[kt * 128:(kt + 1) * 128, ssl])
                    for m in range(6):
                        ps = bps.tile([128, 512], F32, tag="bacc")
                        for kt in range(32):
                            nc.tensor.matmul(
                                ps, wq_sb[:, kt, m * 128:(m + 1) * 128],
                                hid_sb[:, kt, :],
                                start=(kt == 0), stop=(kt == 31))
                        if m < 5:
                            dst = qT[:, m, ssl] if m < 4 else kT[:, ssl]
                            t1 = rt_p.tile([128, 512], F32, tag="t1")
                            t2 = rt_p.tile([128, 512], F32, tag="t2")
                            nc.vector.tensor_mul(t1, ps, cosb[:, ssl])
                            nc.vector.tensor_mul(t2[0:64], ps[64:128], sinb[0:64, ssl])
                            nc.vector.tensor_mul(t2[64:128], ps[0:64], sinb[64:128, ssl])
                            nc.vector.tensor_add(dst, t1, t2)
                        else:
                            nc.vector.tensor_copy(vT[:, ssl], ps)

            # ---------------- Phase C: attention ----------------
            # PSUM budget: scores 2x[128,1024]f32 (4 banks) + transpose
            # 2x[128,8,128]bf16 (2 banks) + pv 2x[128,4,128]f32 (2 banks) = 8
            with tc.tile_pool(name="cps", bufs=2, space="PSUM") as sc_p, \
                 tc.tile_pool(name="tps", bufs=2, space="PSUM") as tr_p, \
                 tc.tile_pool(name="pvs", bufs=2, space="PSUM") as pv_p, \
                 tc.tile_pool(name="pt", bufs=1) as pt_p, \
                 tc.tile_pool(name="psb", bufs=3) as psb_p, \
                 tc.tile_pool(name="dn", bufs=2) as dn_p:
                # V natural tiles from v^T strip
                for g in range(2):
                    tp = tr_p.tile([128, 8, 128], BF16, tag="tp")
                    for i in range(8):
                        st8 = 8 * g + i
                        nc.tensor.transpose(
                            tp[:, i, :], vT[:, st8 * 128:(st8 + 1) * 128], identb)
                    nc.vector.tensor_copy(vnat[:, 8 * g:8 * g + 8, :], tp)

                for t in range(NT):
                    klen = (t + 1) * 128
                    nch = (klen + 1023) // 1024
                    denp = dn_p.tile([128, NQ, 2], F32, tag="denp")
                    recs = dn_p.tile([128, NQ], F32, tag="recs")
                    pts = []
                    for h in range(NQ):
                        pth = pt_p.tile([128, S], BF16, tag=f"pt{h}")
                        pts.append(pth)
                        for ch in range(nch):
                            c0 = ch * 1024
                            cl = min(1024, klen - c0)
                            st = sc_p.tile([128, 1024], F32, tag="sc")
                            for j in range(0, cl, 512):
                                jl = min(512, cl - j)
                                nc.tensor.matmul(
                                    st[:, j:j + jl],
                                    qT[:, h, t * 128:(t + 1) * 128],
                                    kT[:, c0 + j:c0 + j + jl],
                                    start=True, stop=True)
                            if ch == nch - 1:
                                nc.vector.tensor_add(
                                    st[:, cl - 128:cl], st[:, cl - 128:cl], dmask)
                            nc.scalar.activation(
                                pth[:, c0:c0 + cl], st[:, 0:cl], AF.Exp,
                                scale=SCALE, accum_out=denp[:, h, ch:ch + 1])
                    pv = pv_p.tile([128, NQ, 128], F32, tag="pv")
                    for h in range(NQ):
                        pth = pts[h]
                        if nch == 1:
                            nc.vector.reciprocal(recs[:, h:h + 1], denp[:, h, 0:1])
                        else:
                            den = dn_p.tile([128, 1], F32, tag="den")
                            nc.vector.reduce_sum(den, denp[:, h, 0:nch], axis=AX.X)
                            nc.vector.reciprocal(recs[:, h:h + 1], den)
                        nc.vector.tensor_scalar_mul(
                            pth[:, 0:klen], pth[:, 0:klen], recs[:, h:h + 1])
                        nb = t + 1
                        for g0 in range(0, nb, 8):
                            ge = min(8, nb - g0)
                            tp = tr_p.tile([128, 8, 128], BF16, tag="tp")
                            for i in range(ge):
                                kb = g0 + i
                                nc.tensor.transpose(
                                    tp[:, i, :],
                                    pth[:, kb * 128:(kb + 1) * 128], identb)
                            psb = psb_p.tile([128, 8, 128], BF16, tag="psb")
                            nc.vector.tensor_copy(psb[:, 0:ge, :], tp[:, 0:ge, :])
                            for i in range(ge):
                                kb = g0 + i
                                nc.tensor.matmul(
                                    pv[:, h, :], vnat[:, kb, :], psb[:, i, :],
                                    start=(kb == 0), stop=(kb == nb - 1))
                    nc.vector.tensor_copy(
                        attnT[:, :, t * 128:(t + 1) * 128], pv)

            # ---------------- Phase D: o_proj ----------------
            with tc.tile_pool(name="wo", bufs=1) as wo_p, \
                 tc.tile_pool(name="dps", bufs=4, space="PSUM") as dps, \
                 tc.tile_pool(name="ob", bufs=4) as ob_p:
                woT_sb = wo_p.tile([128, NQ, H], BF16, tag="wot")
                for a in range(NQ):
                    nc.sync.dma_start(woT_sb[:, a, :],
                                      woT_d[a * 128:(a + 1) * 128, :])
                for m in range(32):
                    for scc in range(4):
                        po = dps.tile([128, 512], F32, tag="po")
                        for a in range(NQ):
                            nc.tensor.matmul(
                                po, woT_sb[:, a, m * 128:(m + 1) * 128],
                                attnT[:, a, scc * 512:(scc + 1) * 512],
                                start=(a == 0), stop=(a == NQ - 1))
                        ob = ob_p.tile([128, 512], BF16, tag="ob")
                        if (m * 4 + scc) % 2 == 0:
                            nc.vector.tensor_copy(ob, po)
                        else:
                            nc.scalar.copy(ob, po)
                        nc.sync.dma_start(
                            out_d[m * 128:(m + 1) * 128, scc * 512:(scc + 1) * 512],
                            ob)
    nc.compile()
    return nc


def _prep_inputs(hidden_states, position_ids, Wq, Wk, Wv, Wo):
    bf16 = np.dtype(mybir.dt.np(BF16))
    hs = np.asarray(hidden_states, dtype=np.float32)
    hidT = np.ascontiguousarray(hs[0].T).astype(bf16)

    pos = np.asarray(position_ids).reshape(-1).astype(np.float64)
    invf = 1.0 / (10000.0 ** (np.arange(0, 128, 2, dtype=np.float64) / 128.0))
    ang = invf[:, None] * pos[None, :]
    cos_t = np.concatenate([np.cos(ang), np.cos(ang)], axis=0).astype(bf16)
    sin_t = np.concatenate([-np.sin(ang), np.sin(ang)], axis=0).astype(bf16)

    Wq = np.asarray(Wq, dtype=np.float32)
    Wk = np.asarray(Wk, dtype=np.float32)
    Wv = np.asarray(Wv, dtype=np.float32)
    Wo = np.asarray(Wo, dtype=np.float32)
    in_maps = []
    for c in range(NCORES):
        wqkv = np.concatenate([
            Wq[c * 512:(c + 1) * 512],
            Wk[c * 128:(c + 1) * 128],
            Wv[c * 128:(c + 1) * 128]], axis=0)          # [768, H]
        wqkvT = np.ascontiguousarray(wqkv.T).astype(bf16)  # [H, 768]
        woT = np.ascontiguousarray(Wo[:, c * 512:(c + 1) * 512].T).astype(bf16)
        in_maps.append({"hidt": hidT, "wqkvt": wqkvT, "wot": woT,
                        "cos": cos_t, "sin": sin_t})
    return in_maps


def kernel(hidden_states, position_ids, Wq, Wk, Wv, Wo, **extra):
    hs = np.asarray(hidden_states)
    B = hs.shape[0]
    assert B == 1 and hs.shape[1] == S and hs.shape[2] == H

    if "nc" not in _CACHED:
        _CACHED["nc"] = _build_nc()
    nc = _CACHED["nc"]

    in_maps = _prep_inputs(hidden_states, position_ids, Wq, Wk, Wv, Wo)
    res = run_bass_kernel_spmd(nc, in_maps, core_ids=list(range(NCORES)))
    out = np.zeros((H, S), dtype=np.float32)
    for c in range(NCORES):
        out += np.asarray(res.results[c]["outt"]).astype(np.float32)
    return np.ascontiguousarray(out.T).reshape(1, S, H)
